# revision 8
# baseline (speedup 1.0000x reference)
"""Causal self-attention (B=4, S=2048, D=1024, H=16, hd=64) on 8 TRN2 NeuronCores.

Sharding: core c handles batch b = c//2 and head-half g = c%2 (8 heads, 512 of
the 1024 qkv dims).  Each core computes its partial output projection
(x[b] @ Wq_g.T ... attention ... @ Wp_g.T); the host sums the two partials per
batch and adds the bias.

Device kernel (per core), all matmuls in float32r (TF32-like, full-rate):
  P1: Q^T, K^T ([hd, S] layout) and V ([S, hd] layout, with a ones-column per
      head for the softmax denominator) projections from x^T.
  P2: flash-style attention per head-pair chunk: scores computed transposed
      (S^T = K_h @ Q_h^T tiles, [k x q]), exp on ScalarE (scale=1/8 folded in),
      causal masking via gpsimd affine_select on diagonal tiles only, AV
      matmuls accumulate y^T and the softmax denominator (ones column) in
      PSUM; normalization via reciprocal + a small broadcast matmul.
  P3: output projection -> partial out^T, DMA to DRAM.
"""

import numpy as np

B, S, D, H, HD = 4, 2048, 1024, 16, 64
N_CORES = 8
LH = H // 2          # local heads per core (8)
P = 128


def _ensure_concourse():
    try:
        import concourse  # noqa: F401
    except ImportError:
        import sys
        for p in ("/opt/trn_rl_repo", "/root/.axon_site/_ro/trn_rl_repo"):
            if p not in sys.path:
                sys.path.append(p)
        import concourse  # noqa: F401


def build_nc(S_=S, D_=D, LH_=LH, num_devices=N_CORES):
    """Build the per-core Bass program.  Parameterized so a small config can be
    validated in CoreSim.  Requires S_%512==0, D_%128==0, LH_%2==0."""
    _ensure_concourse()
    import concourse.tile as tile
    from concourse import bacc, mybir

    f32 = mybir.dt.float32
    f32r = mybir.dt.float32r
    EXP = mybir.ActivationFunctionType.Exp
    MULT = mybir.AluOpType.mult
    IS_GE = mybir.AluOpType.is_ge

    LHD = LH_ * HD            # local head dims (512)
    NPAIR = LH_ // 2          # head pairs (4)
    DCH = D_ // P             # d contraction chunks (8)
    CH = LHD // P             # hd contraction chunks for out proj (4)
    NQT = S_ // 512           # q tiles (4)
    NKC = S_ // P             # k chunks (16)
    QT = 512                  # q tile width
    KC = P                    # k chunk width

    nc = bacc.Bacc("TRN2", target_bir_lowering=False, debug=False,
                   enable_asserts=True, num_devices=num_devices)

    xT = nc.dram_tensor("xT", [D_, S_], f32, kind="ExternalInput").ap()
    wqT = nc.dram_tensor("wqT", [D_, LHD], f32, kind="ExternalInput").ap()
    wkT = nc.dram_tensor("wkT", [D_, LHD], f32, kind="ExternalInput").ap()
    wvT = nc.dram_tensor("wvT", [D_, LHD], f32, kind="ExternalInput").ap()
    wpT = nc.dram_tensor("wpT", [LHD, D_], f32, kind="ExternalInput").ap()
    outT = nc.dram_tensor("outT", [D_, S_], f32, kind="ExternalOutput").ap()

    xT_r = xT.bitcast(f32r).rearrange("(ko p) s -> p ko s", p=P)
    wqT_r = wqT.bitcast(f32r).rearrange("(ko p) m -> p ko m", p=P)
    wkT_r = wkT.bitcast(f32r).rearrange("(ko p) m -> p ko m", p=P)
    wvT_r = wvT.bitcast(f32r).rearrange("(ko p) m -> p ko m", p=P)
    wpT_r = wpT.bitcast(f32r).rearrange("(co p) d -> p co d", p=P)

    with tile.TileContext(nc) as tc:
        with tc.tile_pool(name="persist", bufs=1) as persist:
            # Persistent SBUF tensors.
            qT = persist.tile([P, NPAIR, S_], f32r, tag="qT")
            kT = persist.tile([P, NPAIR, S_], f32r, tag="kT")
            # v: [s-part, kchunk, head, 64 v-dims + ones col]
            v_sb = persist.tile([P, NKC, LH_, HD + 1], f32r, tag="v")
            # selector for the denominator broadcast matmul (rows 64, 96)
            sel = persist.tile([P, P], f32, tag="sel")
            nc.vector.memset(sel[:], 0.0)
            nc.vector.memset(sel[64:65, 0:64], 1.0)
            nc.vector.memset(sel[96:97, 64:128], 1.0)
            # reciprocal staging rows 64 (head A) / 96 (head B); rows between
            # must be zero so the K=33 broadcast matmul sees 0 contributions
            rt = persist.tile([P, QT], f32, tag="rt")
            nc.vector.memset(rt[64:97, :], 0.0)
            # ones column of v: fill everything with 1.0; the V projection
            # copies overwrite cols 0..63 of each head block, leaving col 64
            nc.vector.memset(v_sb[:].rearrange("p a b c -> p (a b c)").bitcast(f32), 1.0)

            # ---------------- P1: projections ----------------
            with tc.tile_pool(name="xw", bufs=1) as xw, \
                 tc.tile_pool(name="wstr", bufs=2) as wstr, \
                 tc.tile_pool(name="ps1", bufs=2, space="PSUM") as ps1:
                xt = xw.tile([P, DCH, S_], f32r, tag="xt")
                for k in range(DCH):
                    nc.sync.dma_start(xt[:, k, :], xT_r[:, k, :])
                wv_t = xw.tile([P, DCH, LHD], f32r, tag="wv")
                for k in range(DCH):
                    nc.sync.dma_start(wv_t[:, k, :], wvT_r[:, k, :])

                # V projection: out[s-chunk 128, LHD]
                for s in range(NKC):
                    psv = ps1.tile([P, LHD], f32, tag="psv")
                    for k in range(DCH):
                        nc.tensor.matmul(psv[:], xt[:, k, s * P:(s + 1) * P],
                                         wv_t[:, k, :],
                                         start=(k == 0), stop=(k == DCH - 1))
                    # scatter into per-head 65-wide blocks (cols 0..63)
                    nc.vector.tensor_copy(v_sb[:, s, :, 0:HD],
                                          psv.rearrange("p (h d) -> p h d", d=HD))

                # Q^T / K^T projections: out[hd-chunk 128, q-tile 512]
                for a in range(NPAIR):
                    wq_t = wstr.tile([P, DCH, P], f32r, tag="wq")
                    wk_t = wstr.tile([P, DCH, P], f32r, tag="wk")
                    nc.sync.dma_start(wq_t[:], wqT_r[:, :, a * P:(a + 1) * P])
                    nc.sync.dma_start(wk_t[:], wkT_r[:, :, a * P:(a + 1) * P])
                    for n in range(NQT):
                        psq = ps1.tile([P, QT], f32, tag="psq")
                        psk = ps1.tile([P, QT], f32, tag="psk")
                        for k in range(DCH):
                            nc.tensor.matmul(psq[:], wq_t[:, k, :],
                                             xt[:, k, n * QT:(n + 1) * QT],
                                             start=(k == 0), stop=(k == DCH - 1))
                        for k in range(DCH):
                            nc.tensor.matmul(psk[:], wk_t[:, k, :],
                                             xt[:, k, n * QT:(n + 1) * QT],
                                             start=(k == 0), stop=(k == DCH - 1))
                        nc.vector.tensor_copy(qT[:, a, n * QT:(n + 1) * QT], psq[:])
                        nc.vector.tensor_copy(kT[:, a, n * QT:(n + 1) * QT], psk[:])

            # ---------------- P2 + P3 ----------------
            with tc.tile_pool(name="ysb", bufs=1) as ysb, \
                 tc.tile_pool(name="ppool", bufs=3) as ppool, \
                 tc.tile_pool(name="small", bufs=2) as small, \
                 tc.tile_pool(name="ostg", bufs=2) as ostg, \
                 tc.tile_pool(name="ps_s", bufs=2, space="PSUM") as ps_s, \
                 tc.tile_pool(name="ps_y", bufs=2, space="PSUM") as ps_y, \
                 tc.tile_pool(name="ps_m", bufs=1, space="PSUM") as ps_m:
                yT = ysb.tile([P, CH, S_], f32r, tag="yT")
                wp_t = ysb.tile([P, CH, D_], f32r, tag="wp")
                for c in range(CH):
                    nc.sync.dma_start(wp_t[:, c, :], wpT_r[:, c, :])

                for j in range(NQT):
                    kcount = 4 * j + 4
                    for a in range(NPAIR):
                        psA = ps_y.tile([P, QT], f32, tag="psy")
                        psB = ps_y.tile([P, QT], f32, tag="psy")
                        for kc in range(kcount):
                            ss = ps_s.tile([P, 2 * QT], f32, tag="ss")
                            for h2 in range(2):
                                o = 64 * h2
                                nc.tensor.matmul(
                                    ss[:, h2 * QT:(h2 + 1) * QT],
                                    kT[o:o + 64, a, kc * KC:(kc + 1) * KC],
                                    qT[o:o + 64, a, j * QT:(j + 1) * QT],
                                    start=True, stop=True)
                            pt = ppool.tile([P, 2 * QT], f32r, tag="pt")
                            nc.scalar.activation(pt[:], ss[:], EXP, scale=0.125)
                            r = kc - 4 * j
                            if r >= 0:  # diagonal tile: causal mask
                                for h2 in range(2):
                                    nc.gpsimd.affine_select(
                                        out=pt[:, h2 * QT:(h2 + 1) * QT],
                                        in_=pt[:, h2 * QT:(h2 + 1) * QT],
                                        pattern=[[1, QT]],
                                        compare_op=IS_GE,
                                        fill=0.0,
                                        base=-KC * r,
                                        channel_multiplier=-1)
                            for h2, psy in ((0, psA), (1, psB)):
                                nc.tensor.matmul(
                                    psy[0:HD + 1, :],
                                    v_sb[:, kc, 2 * a + h2, :],
                                    pt[:, h2 * QT:(h2 + 1) * QT],
                                    start=(kc == 0), stop=(kc == kcount - 1))
                        # normalization (plain reciprocal; approx_fast is
                        # broken on HW in this env, and recip can't read PSUM)
                        nc.vector.tensor_copy(rt[64:65, :], psA[HD:HD + 1, :])
                        nc.vector.tensor_copy(rt[96:97, :], psB[HD:HD + 1, :])
                        nc.vector.reciprocal(rt[64:65, :], rt[64:65, :])
                        nc.vector.reciprocal(rt[96:97, :], rt[96:97, :])
                        bc = ps_m.tile([P, QT], f32, tag="bc")
                        nc.tensor.matmul(bc[:], sel[64:97, :], rt[64:97, :],
                                         start=True, stop=True)
                        bcs = small.tile([P, QT], f32, tag="bcs")
                        nc.vector.tensor_copy(bcs[:], bc[:])
                        nc.vector.tensor_tensor(
                            yT[0:64, a, j * QT:(j + 1) * QT],
                            psA[0:HD, :], bcs[0:64, :], MULT)
                        nc.vector.tensor_tensor(
                            yT[64:128, a, j * QT:(j + 1) * QT],
                            psB[0:HD, :], bcs[64:128, :], MULT)

                    # P3 for this q tile
                    for m in range(DCH):
                        po = ps_m.tile([P, QT], f32, tag="po")
                        for c in range(CH):
                            nc.tensor.matmul(po[:], wp_t[:, c, m * P:(m + 1) * P],
                                             yT[:, c, j * QT:(j + 1) * QT],
                                             start=(c == 0), stop=(c == CH - 1))
                        ot = ostg.tile([P, QT], f32, tag="ot")
                        nc.vector.tensor_copy(ot[:], po[:])
                        nc.sync.dma_start(outT[m * P:(m + 1) * P,
                                               j * QT:(j + 1) * QT], ot[:])

    nc.compile()
    return nc


class _Runner:
    """Compile once; execute the SPMD program on 8 cores via PJRT."""

    def __init__(self):
        _ensure_concourse()
        import jax
        import numpy as _np
        from jax.sharding import Mesh, PartitionSpec
        from jax.experimental.shard_map import shard_map
        from concourse import bass2jax, mybir

        self.nc = build_nc()
        bass2jax.install_neuronx_cc_hook()
        nc = self.nc

        partition_name = (nc.partition_id_tensor.name
                          if nc.partition_id_tensor else None)
        in_names, out_names, out_avals, zero_shapes = [], [], [], []
        for alloc in nc.m.functions[0].allocations:
            if not isinstance(alloc, mybir.MemoryLocationSet):
                continue
            name = alloc.memorylocations[0].name
            if alloc.kind == "ExternalInput":
                if name != partition_name:
                    in_names.append(name)
            elif alloc.kind == "ExternalOutput":
                out_names.append(name)
                shape = tuple(alloc.tensor_shape)
                dtype = mybir.dt.np(alloc.dtype)
                out_avals.append(jax.core.ShapedArray(shape, dtype))
                zero_shapes.append((shape, dtype))
        self.in_names, self.out_names = in_names, out_names
        self.out_avals, self.zero_shapes = out_avals, zero_shapes
        n_params, n_outs = len(in_names), len(out_names)

        all_in_names = in_names + out_names
        if partition_name is not None:
            all_in_names = all_in_names + [partition_name]

        def _body(*args):
            operands = list(args)
            if partition_name is not None:
                operands.append(bass2jax.partition_id_tensor())
            outs = bass2jax._bass_exec_p.bind(
                *operands,
                out_avals=tuple(out_avals),
                in_names=tuple(all_in_names),
                out_names=tuple(out_names),
                lowering_input_output_aliases=(),
                sim_require_finite=True,
                sim_require_nnan=True,
                nc=nc,
            )
            return tuple(outs)

        devices = jax.devices()[:N_CORES]
        mesh = Mesh(_np.asarray(devices), ("core",))
        donate = tuple(range(n_params, n_params + n_outs))
        self._sharded = jax.jit(
            shard_map(_body, mesh=mesh,
                      in_specs=(PartitionSpec("core"),) * (n_params + n_outs),
                      out_specs=(PartitionSpec("core"),) * n_outs,
                      check_rep=False),
            donate_argnums=donate, keep_unused=True)

    def __call__(self, in_maps):
        import numpy as _np
        concat_in = [
            _np.concatenate([in_maps[c][name] for c in range(N_CORES)], axis=0)
            for name in self.in_names
        ]
        concat_zeros = [
            _np.zeros((N_CORES * s[0], *s[1:]), dt) for s, dt in self.zero_shapes
        ]
        out_arrs = self._sharded(*concat_in, *concat_zeros)
        return [
            {name: _np.asarray(out_arrs[i]).reshape(N_CORES, *self.out_avals[i].shape)[c]
             for i, name in enumerate(self.out_names)}
            for c in range(N_CORES)
        ]


_RUNNER = None


def _get_runner():
    global _RUNNER
    if _RUNNER is None:
        _RUNNER = _Runner()
    return _RUNNER


def shard_inputs(x, Wq, Wk, Wv, Wp):
    """Full inputs -> per-core input maps (host-side layout prep)."""
    in_maps = []
    for c in range(N_CORES):
        b, g = c // 2, c % 2
        sl = slice(g * LH * HD, (g + 1) * LH * HD)
        in_maps.append({
            "xT": np.ascontiguousarray(x[b].T),
            "wqT": np.ascontiguousarray(Wq[sl, :].T),
            "wkT": np.ascontiguousarray(Wk[sl, :].T),
            "wvT": np.ascontiguousarray(Wv[sl, :].T),
            "wpT": np.ascontiguousarray(Wp[:, sl].T),
        })
    return in_maps


def kernel(x, Wq, Wk, Wv, Wp, bp):
    x = np.asarray(x, dtype=np.float32)
    Wq = np.asarray(Wq, dtype=np.float32)
    Wk = np.asarray(Wk, dtype=np.float32)
    Wv = np.asarray(Wv, dtype=np.float32)
    Wp = np.asarray(Wp, dtype=np.float32)
    bp = np.asarray(bp, dtype=np.float32)

    runner = _get_runner()
    outs = runner(shard_inputs(x, Wq, Wk, Wv, Wp))
    out = np.empty((B, S, D), np.float32)
    for b in range(B):
        out[b] = outs[2 * b]["outT"].T + outs[2 * b + 1]["outT"].T + bp
    return out
